# revision 22
# baseline (speedup 1.0000x reference)
import sys
if "/opt/trn_rl_repo" not in sys.path:
    sys.path.insert(0, "/opt/trn_rl_repo")

import numpy as np
import ml_dtypes
import concourse.bacc as bacc
import concourse.tile as tile
from concourse import mybir
from concourse.bass_utils import run_bass_kernel_spmd

B, S, D = 4, 2048, 1024
NCORES = 8
F32 = mybir.dt.float32
F32R = mybir.dt.float32r
BF16 = mybir.dt.bfloat16
BF = ml_dtypes.bfloat16
_cache = {}

# Per-core tensor-parallel MHA: each core owns 128 of the 1024 qkv dims
# (= 2 heads x 64), computes its heads' attention for all 4 batches, and
# produces a full [D, S] partial of the output projection which the host
# sums across cores.  All matmuls in bf16 (fp32 PSUM accumulate).
#
# Head0 lives on SBUF partitions 0-63, head1 on 64-127, so the 64-deep
# score matmuls auto-derive PE row tiles (0,0)/(64,0) and execute
# concurrently when issued back-to-back.  Stages are processed two at a
# time (4 row-tiled score MMs, then 4 full-array PV MMs) to minimize PE
# tiling-mode switches, which drain the systolic array.


def _build():
    if "nc" in _cache:
        return _cache["nc"]
    nc = bacc.Bacc()
    xt = nc.dram_tensor("xt", [D, B * S], BF16, kind="ExternalInput")
    wq = nc.dram_tensor("wq", [128, D], BF16, kind="ExternalInput")
    wk = nc.dram_tensor("wk", [128, D], BF16, kind="ExternalInput")
    wv = nc.dram_tensor("wv", [128, D], BF16, kind="ExternalInput")
    wo = nc.dram_tensor("wo", [128, D], BF16, kind="ExternalInput")
    bq = nc.dram_tensor("bq", [128, 1], F32, kind="ExternalInput")
    bk = nc.dram_tensor("bk", [128, 1], F32, kind="ExternalInput")
    bv = nc.dram_tensor("bv", [128, 1], F32, kind="ExternalInput")
    idm = nc.dram_tensor("idm", [128, 128], F32R, kind="ExternalInput")
    on32 = nc.dram_tensor("on32", [128, 32], BF16, kind="ExternalInput")
    po = nc.dram_tensor("po", [B * D, S], BF16, kind="ExternalOutput")

    ACT = mybir.ActivationFunctionType

    with tile.TileContext(nc) as tc:
        with tc.tile_pool(name="sb", bufs=1) as sb, \
             tc.tile_pool(name="ps", bufs=1, space="PSUM") as ps:
            wq_sb = sb.tile([128, D], BF16)
            wk_sb = sb.tile([128, D], BF16)
            wv_sb = sb.tile([128, D], BF16)
            wo_sb = sb.tile([128, D], BF16)
            bq_sb = sb.tile([128, 1], F32)
            bk_sb = sb.tile([128, 1], F32)
            bv_sb = sb.tile([128, 1], F32)
            ident = sb.tile([128, 128], F32R)
            nb2 = sb.tile([128, 1], F32)
            nc.vector.memset(nb2[:, :], -4.5)
            # V (+ its bias) first: the prologue computes V before Q/K
            nc.sync.dma_start(out=wv_sb, in_=wv[:, :])
            nc.sync.dma_start(out=bv_sb, in_=bv[:, :])
            nc.sync.dma_start(out=wq_sb, in_=wq[:, :])
            nc.sync.dma_start(out=bq_sb, in_=bq[:, :])
            nc.sync.dma_start(out=wk_sb, in_=wk[:, :])
            nc.sync.dma_start(out=bk_sb, in_=bk[:, :])
            nc.sync.dma_start(out=ident, in_=idm[:, :])
            nc.sync.dma_start(out=wo_sb, in_=wo[:, :])

            qt = [sb.tile([128, S], BF16, name=f"qt{i}") for i in range(2)]
            kt = [sb.tile([128, S], BF16, name=f"kt{i}") for i in range(2)]
            vt = [sb.tile([128, S], F32R, name=f"vt{i}") for i in range(2)]
            # vp: per k-tile 130 cols = 64 V_h0 | ones | 64 V_h1 | ones
            vp = [sb.tile([128, 16 * 130], BF16, name=f"vp{i}") for i in range(2)]
            nc.sync.dma_start(out=vp[0][:, 64:16 * 130:65], in_=on32[:, :])
            nc.sync.dma_start(out=vp[1][:, 64:16 * 130:65], in_=on32[:, :])
            ctxT = [sb.tile([128, S], BF16, name=f"ctxT{i}") for i in range(2)]

            # warm-up: keep PE busy while the first xs tiles stream in, so
            # the HAM clock-gate opens before the real work starts
            for i in range(20):
                wup = ps.tile([128, 1024], F32, tag="sc", bufs=2, name="wup")
                nc.tensor.matmul(wup[:, 0:512], wq_sb[:, 0:128],
                                 wq_sb[:, 0:512], start=True, stop=True)

            def emit_xs_chunk(xsl, bi, split, c):
                # batch 0's even k-tiles stream on the idle gpsimd DMA
                # queue so they don't all wait behind the sync queue;
                # later batches use sync (gpsimd then carries broadcasts)
                w = S // split
                for k in range(8):
                    eng = nc.gpsimd if (bi == 0 and k % 2 == 0) \
                        else nc.sync
                    eng.dma_start(
                        out=xsl[k][:, c * w:(c + 1) * w],
                        in_=xt[k * 128:(k + 1) * 128,
                               bi * S + c * w:bi * S + (c + 1) * w])

            def emit_xs(bi, split=1, cols=None):
                xsl = [sb.tile([128, S], BF16, tag="xs", bufs=24,
                               name="xs") for _ in range(8)]
                for c in (range(split) if cols is None else cols):
                    emit_xs_chunk(xsl, bi, split, c)
                return xsl

            def qk_unit(proj, ct, xsl, par):
                # 512 token-cols of Q, K or V
                wt = (wq_sb, wk_sb, wv_sb)[proj]
                bt = (bq_sb, bk_sb, bv_sb)[proj]
                pq = ps.tile([128, 512], F32, tag="mx", bufs=2)
                for k in range(8):
                    nc.tensor.matmul(
                        pq, wt[:, k * 128:(k + 1) * 128],
                        xsl[k][:, ct * 512:(ct + 1) * 512],
                        start=(k == 0), stop=(k == 7))
                dst = (qt, kt, vt)[proj][par][:, ct * 512:(ct + 1) * 512]
                with nc.allow_low_precision(reason="bf16 qkv"):
                    nc.vector.tensor_scalar_add(dst, pq, bt[:, 0:1])

            def t4_unit(j, par):
                # V' k-tiles 4j..4j+3 via PE transpose, grouped so the
                # transpose-mode switch is paid once per four
                for t in range(4 * j, 4 * j + 4):
                    tp = ps.tile([128, 128], F32R, tag="mx", bufs=2,
                                 name="tp")
                    nc.tensor.transpose(
                        tp[:, :], vt[par][:, t * 128:(t + 1) * 128],
                        ident[:, :])
                    nc.vector.tensor_copy(
                        out=vp[par][:, t * 130:t * 130 + 64], in_=tp[:, 0:64])
                    nc.vector.tensor_copy(
                        out=vp[par][:, t * 130 + 65:t * 130 + 129],
                        in_=tp[:, 64:128])

            def pso_unit(m, c2, b, par, use_act=False, dma_eng=None):
                # out-proj partial: po[b, m-tile, 512 tok] from ctxT[par]
                pso = ps.tile([128, 512], F32, tag="mx", bufs=2)
                nc.tensor.matmul(
                    pso, wo_sb[:, m * 128:(m + 1) * 128],
                    ctxT[par][:, c2 * 512:(c2 + 1) * 512],
                    start=True, stop=True)
                ob = sb.tile([128, 512], BF16, tag="ob", bufs=6)
                if use_act:
                    with nc.allow_low_precision(reason="bf16 out"):
                        nc.scalar.activation(out=ob, in_=pso,
                                             func=ACT.Identity, scale=1.0)
                else:
                    nc.vector.tensor_copy(out=ob, in_=pso)
                (dma_eng or nc.sync).dma_start(
                    out=po[b * D + m * 128:b * D + (m + 1) * 128,
                           c2 * 512:(c2 + 1) * 512],
                    in_=ob)

            def run_unit(u, xsl_by_par):
                if u[0] == "qk":
                    qk_unit(u[1], u[2], xsl_by_par[u[3]], u[3])
                elif u[0] == "t4":
                    t4_unit(u[1], u[2])
                else:
                    _, m, c2, b, p, ua = u
                    pso_unit(m, c2, b, p, use_act=ua)

            # group g covers stages 2g, 2g+1; stage s1 = (Q, t) does both
            # heads' scores for q-block Q (512 tok) x k-tile t (128 tok)
            def emit_scores(s1, par, scp):
                Q, t = s1 // 16, s1 % 16
                for h in range(2):
                    nc.tensor.matmul(
                        scp[:, h * 512:(h + 1) * 512],
                        kt[par][h * 64:(h + 1) * 64, t * 128:(t + 1) * 128],
                        qt[par][h * 64:(h + 1) * 64, Q * 512:(Q + 1) * 512],
                        start=True, stop=True)

            def emit_pv(s1, par, et, cxs):
                Q, t = s1 // 16, s1 % 16
                if t == 0:
                    cxs[0] = ps.tile([128, 512], F32, tag="cx", bufs=2,
                                     name="cx0")
                    cxs[1] = ps.tile([128, 512], F32, tag="cx", bufs=2,
                                     name="cx1")
                for h in range(2):
                    nc.tensor.matmul(
                        cxs[h][0:65, :],
                        vp[par][:, t * 130 + h * 65:t * 130 + (h + 1) * 65],
                        et[:, h * 512:(h + 1) * 512],
                        start=(t == 0), stop=(t == 15))

            def finalize_a(Q, cxs, pend, par, use_scalar=True):
                # evacuate PSUM fast (releases cx for next Q).  h0 copy on
                # vector, h1 copy on scalar so the two drain in parallel
                # (except on the ACT-bound last batch: keep ACT clear).
                for h in range(2):
                    cxs_sb = sb.tile([64, 512], F32, tag="cxs", bufs=4)
                    den = sb.tile([1, 512], F32, tag="den", bufs=4)
                    if h == 1 and use_scalar:
                        nc.scalar.activation(out=cxs_sb, in_=cxs[h][0:64, :],
                                             func=ACT.Identity, scale=1.0)
                        nc.scalar.activation(out=den[0:1, :],
                                             in_=cxs[h][64:65, :],
                                             func=ACT.Identity, scale=1.0)
                    else:
                        nc.vector.tensor_copy(out=cxs_sb, in_=cxs[h][0:64, :])
                        nc.vector.tensor_copy(out=den[0:1, :],
                                              in_=cxs[h][64:65, :])
                    rcf = sb.tile([1, 512], F32, tag="rcf", bufs=4)
                    nc.vector.reciprocal_approx_fast(
                        out=rcf[0:1, :], in_=den[0:1, :])
                    pend.append((h, Q, cxs_sb, rcf, par))

            def finalize_b(item):
                # broadcast 1/denom across 64 partitions (idle gpsimd engine)
                h, Q, cxs_sb, rcf, par = item
                c0 = Q * 512
                bcs = sb.tile([64, 512], F32, tag="bcs", bufs=3)
                nc.gpsimd.partition_broadcast(bcs[:, :], rcf[0:1, :])
                nc.vector.tensor_tensor(
                    ctxT[par][h * 64:(h + 1) * 64, c0:c0 + 512],
                    cxs_sb[0:64, :], bcs[:, :], mybir.AluOpType.mult)

            # ── batch 0 mini-prologue: just enough for the first groups;
            # the rest interleaves into batch 0's stage loop ──
            xsl_by_par = {0: emit_xs(0, split=4), 1: None}
            qk_unit(2, 0, xsl_by_par[0], 0)  # V ct0
            qk_unit(0, 0, xsl_by_par[0], 0)  # Q ct0
            qk_unit(1, 0, xsl_by_par[0], 0)  # K ct0
            t4_unit(0, 0)

            pend = []
            for i in range(B):
                b, par = i, i % 2
                # units with an earliest-group eligibility (batch 0's own
                # remaining QKV, paced to DMA arrival and first use)
                # every batch computes its own Q1-3 projection late (its
                # q-block Q=ct is first needed at stage s=16*ct), shrinking
                # the cross-batch unit load
                elig = [(2, ("qk", 0, 1, par)), (18, ("qk", 0, 2, par)),
                        (34, ("qk", 0, 3, par))]
                if i == 0:
                    elig = [(0, ("qk", 2, 1, 0)), (0, ("qk", 1, 1, 0)),
                            (2, ("t4", 1, 0)), (2, ("qk", 0, 1, 0)),
                            (4, ("qk", 2, 2, 0)), (4, ("qk", 1, 2, 0)),
                            (6, ("t4", 2, 0)),
                            (8, ("qk", 2, 3, 0)), (8, ("qk", 1, 3, 0)),
                            (10, ("t4", 3, 0)),
                            (18, ("qk", 0, 2, 0)), (34, ("qk", 0, 3, 0))]
                if i < B - 1:
                    npar = 1 - par
                    xsl_by_par[npar] = emit_xs(i + 1)
                    qi = [("qk", 2, 0, npar), ("qk", 0, 0, npar),
                          ("qk", 1, 0, npar), ("t4", 0, npar),
                          ("qk", 2, 1, npar), ("qk", 1, 1, npar),
                          ("t4", 1, npar),
                          ("qk", 2, 2, npar), ("qk", 1, 2, npar),
                          ("t4", 2, npar),
                          ("qk", 2, 3, npar), ("qk", 1, 3, npar),
                          ("t4", 3, npar)]
                else:
                    qi = []
                # previous batch's out-proj fills this batch's PE slack;
                # b1's q-blocks 2-3 ride in the ACT-bound last batch
                if i == 2:
                    oi = [("o", m, c2, 1, 1, False)
                          for m in range(8) for c2 in range(2)]
                elif i == 3:
                    oi = ([("o", m, c2, 1, 1, False)
                           for m in range(8) for c2 in (2, 3)]
                          + [("o", m, c2, 2, 0, False)
                             for m in range(8) for c2 in range(4)])
                elif i > 0:
                    oi = [("o", m, c2, b - 1, 1 - par, False)
                          for m in range(8) for c2 in range(4)]
                else:
                    oi = []
                # last batch: its own q-block 0-2 out-proj rides the tail
                tail = ([("o", m, c2, b, par, False)
                         for c2 in range(3) for m in range(8)]
                        if i == B - 1 else [])
                units = []
                while qi or oi:
                    if qi:
                        units.append(qi.pop(0))
                    if qi:
                        units.append(qi.pop(0))
                    if oi:
                        units.append(oi.pop(0))
                nunits = len(units)
                done = 0
                tdone = 0
                edone = 0
                ets = {}
                cxs = {}

                for s in range(0, 64, 2):
                    for s1 in (s, s + 1):
                        scp = ps.tile([128, 1024], F32, tag="sc", bufs=2)
                        emit_scores(s1, par, scp)
                        et = sb.tile([128, 1024], BF16, tag="et", bufs=6)
                        nc.scalar.activation(out=et, in_=scp, func=ACT.Exp,
                                             scale=0.125)
                        ets[s1] = et
                    if s >= 2:
                        for s1 in (s - 2, s - 1):
                            emit_pv(s1, par, ets.pop(s1), cxs)
                            if s1 % 16 == 15:
                                finalize_a(s1 // 16, cxs, pend, par,
                                           use_scalar=(i < B - 1))
                    if s % 16 == 4 and pend:
                        for item in pend:
                            finalize_b(item)
                        pend = []
                    # eligibility-gated units (batch 0 prologue remainder)
                    while edone < len(elig) and elig[edone][0] <= s:
                        run_unit(elig[edone][1], xsl, i)
                        edone += 1
                    # no units in a batch's first two groups: a unit MM
                    # waiting on fresh DMAs at the front of the PE FIFO
                    # head-of-line-blocks the ready score MMs behind it
                    want = 0 if s < 4 else (s - 2) * nunits // 60
                    while done < want:
                        run_unit(units[done], xsl, i)
                        done += 1
                    # 8 out-proj units per finished q-block; q-block Q's
                    # ctxT is written by the pend flush at s = 16*Q+20
                    twant = sum(max(0, min(s - (16 * c2 + 19), 8))
                                for c2 in range(3))
                    while tdone < min(twant, len(tail)):
                        run_unit(tail[tdone], xsl, i)
                        tdone += 1
                for s1 in (62, 63):
                    emit_pv(s1, par, ets.pop(s1), cxs)
                finalize_a(3, cxs, pend, par)

            for item in pend:
                finalize_b(item)
            # epilogue: out-proj q-block 3 of last batch
            for m in range(8):
                pso_unit(m, 3, B - 1, (B - 1) % 2, use_act=(m % 2 == 1))
    nc.finalize()
    _cache["nc"] = nc
    return nc


def _in_maps(x, qkv_w, qkv_b, out_w):
    xT = np.ascontiguousarray(
        x.reshape(B * S, D).T).astype(BF)
    on32 = np.ones((128, 32), dtype=BF)
    idm = np.eye(128, dtype=np.float32)
    in_maps = []
    for c in range(NCORES):
        base = c * 128

        def warr(Wn):
            # W [128 out, 1024 in] -> lhsT chunks [128 in-part, 8k x 128 out]
            return np.ascontiguousarray(
                Wn.reshape(128, 8, 128).transpose(2, 1, 0).reshape(128, 1024)
            ).astype(BF)

        V = out_w[:, base:base + 128]  # [1024 out, 128 ctx]
        in_maps.append({
            "xt": xT,
            "wq": warr(qkv_w[base:base + 128, :]),
            "wk": warr(qkv_w[D + base:D + base + 128, :]),
            "wv": warr(qkv_w[2 * D + base:2 * D + base + 128, :]),
            "wo": np.ascontiguousarray(
                V.reshape(8, 128, 128).transpose(2, 0, 1).reshape(128, 1024)
            ).astype(BF),
            "bq": qkv_b[base:base + 128].reshape(128, 1).astype(np.float32),
            "bk": qkv_b[D + base:D + base + 128].reshape(128, 1).astype(np.float32),
            "bv": qkv_b[2 * D + base:2 * D + base + 128].reshape(128, 1).astype(np.float32),
            "idm": idm,
            "on32": on32,
        })
    return in_maps


def kernel(x, qkv_w, qkv_b, out_w, out_b):
    nc = _build()
    in_maps = _in_maps(x, qkv_w, qkv_b, out_w)
    res = run_bass_kernel_spmd(nc, in_maps, core_ids=list(range(NCORES)),
                               trace=False)
    kernel.last_exec_ns = res.exec_time_ns
    acc = np.zeros((B, D, S), dtype=np.float32)
    for c in range(NCORES):
        acc += res.results[c]["po"].astype(np.float32).reshape(B, D, S)
    out = acc.transpose(0, 2, 1) + out_b.astype(np.float32)
    return out.astype(np.float32)
